# revision 8
# baseline (speedup 1.0000x reference)
"""Trainium2 Bass kernel for BinaryLinear: out = x @ sign(W).T

Shapes (hardcoded): x [32768, 2048] f32, weight [2048, 2048] f32,
out [32768, 2048] f32.

Strategy: data-parallel over 8 NeuronCores — shard the token axis
(4096 tokens/core), replicate the weight. Per core:
  - weight prep: f32 loads on the Scalar HWDGE queue interleaved with
    Sign activations (f32 -> bf16), then batched xbar DMA-transposes
    into the resident swT[i, ot, ic, o] (contiguous per-ot writes).
  - x pipeline: f32 HWDGE loads, DVE cast f32 -> bf16, batched xbar
    DMA-transpose -> xT[i, ic, t].
  - matmuls run in 8-token-tile blocks x 4 output-chunk waves so the
    first waves depend only on the first weight tiles: out[128t, 512o]
    accumulates 16 bf16 matmuls (xT chunk stationary, swT moving) in
    PSUM, DVE copies PSUM->SBUF, per-chunk DMA stores. Next-block
    loads/casts/transposes are emitted between waves so every in-order
    engine stream stays ahead of the tensor engine.

All xbar transposes issue from the Sync engine only — concurrent
DMA_TRANSPOSE from two engines corrupts the shared xbar. SWDGE
(GpSimd) DMA is avoided entirely: its casting path moves only
~80 GB/s. The tensor engine runs just the 2048 N=512 matmuls
(~213 ns each warm).
"""

import sys

if "/opt/trn_rl_repo" not in sys.path:
    sys.path.insert(0, "/opt/trn_rl_repo")

import numpy as np

T, I, O = 32768, 2048, 2048
NCORES = 8
TL = T // NCORES  # tokens per core

_NC = None


def _build():
    import concourse.bacc as bacc
    import concourse.mybir as mybir
    from concourse import tile
    from contextlib import ExitStack

    f32 = mybir.dt.float32
    bf16 = mybir.dt.bfloat16

    IC = I // 128  # i-chunks (contraction)
    OT = O // 128  # weight row tiles
    NT = TL // 128  # token tiles per core
    OCW = 512  # matmul moving free dim
    NOC = O // OCW
    BLK = 8  # token tiles per block
    NBLK = NT // BLK

    nc = bacc.Bacc("TRN2", target_bir_lowering=False, debug=False, num_devices=NCORES)
    x = nc.dram_tensor("x", [TL, I], f32, kind="ExternalInput")
    w = nc.dram_tensor("weight", [O, I], f32, kind="ExternalInput")
    out = nc.dram_tensor("out", [TL, O], f32, kind="ExternalOutput")

    with tile.TileContext(nc) as tc, ExitStack() as ctx:
        # sign(W).T resident: swT[i_p, ot, ic, o_l] =
        # sign(W)[128*ot + o_l, 128*ic + i_p]
        swt_pool = ctx.enter_context(tc.tile_pool(name="swt", bufs=1))
        swT = swt_pool.tile([128, OT, IC, 128], bf16)

        wprep = ctx.enter_context(tc.tile_pool(name="wprep", bufs=1))
        w_f32 = [
            wprep.tile([128, I], f32, tag="w_f32", name=f"w_f32_{ot}", bufs=4)
            for ot in range(OT)
        ]
        w_sgn = [
            wprep.tile([128, I], bf16, tag="w_sgn", name=f"w_sgn_{ot}", bufs=3)
            for ot in range(OT)
        ]
        # interleave loads and signs so the in-order Scalar stream never
        # waits on a pool slot whose release is behind it
        for ot in range(4):
            nc.scalar.dma_start(w_f32[ot][:], w[128 * ot : 128 * (ot + 1), :])
        for ot in range(OT):
            nc.scalar.activation(
                w_sgn[ot][:], w_f32[ot][:], mybir.ActivationFunctionType.Sign
            )
            if ot + 4 < OT:
                nc.scalar.dma_start(
                    w_f32[ot + 4][:], w[128 * (ot + 4) : 128 * (ot + 5), :]
                )

        xpool = ctx.enter_context(tc.tile_pool(name="xpool", bufs=1))
        xtpool = ctx.enter_context(tc.tile_pool(name="xtpool", bufs=10))
        opool = ctx.enter_context(tc.tile_pool(name="opool", bufs=6))
        psum_mm = ctx.enter_context(tc.tile_pool(name="psum_mm", bufs=4, space="PSUM"))

        x_f32 = [None] * NT
        x_bf = [None] * NT
        xT = [None] * NT

        def load_x(tt, eng):
            x_f32[tt] = xpool.tile(
                [128, I], f32, tag="x_f32", name=f"x_f32_{tt}", bufs=3
            )
            eng.dma_start(x_f32[tt][:], x[128 * tt : 128 * (tt + 1), :])

        def cast_x(tt):
            x_bf[tt] = xpool.tile(
                [128, I], bf16, tag="x_bf", name=f"x_bf_{tt}", bufs=3
            )
            nc.vector.tensor_copy(x_bf[tt][:], x_f32[tt][:])

        def transpose_x(tt):
            xT[tt] = xtpool.tile([128, IC, 128], bf16, tag="xT", name=f"xT_{tt}")
            nc.sync.dma_start(xT[tt][:], x_bf[tt][:], transpose=True)

        # prologue: block-0 x tiles; w transposes interleaved with x
        # transposes so neither chain stalls the other on the Sync engine
        for tt in range(BLK):
            load_x(tt, nc.sync)
        for tt in range(BLK):
            cast_x(tt)
        for k in range(4):
            nc.sync.dma_start(swT[:, k, :, :], w_sgn[k][:], transpose=True)
            transpose_x(k)
        for tt in range(4, BLK):
            transpose_x(tt)
        for ot in range(4, OT):
            nc.sync.dma_start(swT[:, ot, :, :], w_sgn[ot][:], transpose=True)

        for blk in range(NBLK):
            tts = range(blk * BLK, (blk + 1) * BLK)
            nxt = range((blk + 1) * BLK, (blk + 2) * BLK) if blk + 1 < NBLK else []
            for oc in range(NOC):
                for tt in tts:
                    acc = psum_mm.tile(
                        [128, OCW], f32, tag="acc", name=f"acc_{tt}_{oc}"
                    )
                    for ic in range(IC):
                        nc.tensor.matmul(
                            acc[:],
                            xT[tt][:, ic, :],
                            swT[:, 4 * oc : 4 * (oc + 1), ic, :],
                            start=(ic == 0),
                            stop=(ic == IC - 1),
                        )
                    o_ch = opool.tile([128, OCW], f32, tag="o_ch")
                    nc.vector.tensor_copy(o_ch[:], acc[:])
                    nc.sync.dma_start(
                        out[128 * tt : 128 * (tt + 1), OCW * oc : OCW * (oc + 1)],
                        o_ch[:],
                    )
                # prefetch next block between waves (Scalar queue is idle
                # after weight prep; Sync handles the transposes)
                if oc == 0:
                    for tt in nxt:
                        load_x(tt, nc.scalar)
                elif oc == 1:
                    for tt in nxt:
                        cast_x(tt)
                elif oc == 2:
                    for tt in nxt:
                        transpose_x(tt)

    nc.compile()
    return nc


def _get_nc():
    global _NC
    if _NC is None:
        _NC = _build()
    return _NC


def _in_maps(x, w):
    x = np.ascontiguousarray(np.asarray(x, dtype=np.float32))
    w = np.ascontiguousarray(np.asarray(w, dtype=np.float32))
    assert x.shape == (T, I) and w.shape == (O, I)
    return [
        {"x": x[c * TL : (c + 1) * TL], "weight": w} for c in range(NCORES)
    ]


def kernel(**inputs):
    from concourse.bass_utils import run_bass_kernel_spmd

    nc = _get_nc()
    res = run_bass_kernel_spmd(
        nc, _in_maps(inputs["x"], inputs["weight"]), core_ids=list(range(NCORES))
    )
    return np.concatenate([r["out"] for r in res.results], axis=0)


# revision 9
# speedup vs baseline: 1.0805x; 1.0805x over previous
"""Trainium2 Bass kernel for BinaryLinear: out = x @ sign(W).T

Shapes (hardcoded): x [32768, 2048] f32, weight [2048, 2048] f32,
out [32768, 2048] f32.

Strategy: data-parallel over 8 NeuronCores — shard the token axis
(4096 tokens/core), replicate the weight. Per core:
  - weight prep: f32 loads on the Scalar HWDGE queue interleaved with
    Sign activations (f32 -> bf16), then batched xbar DMA-transposes
    into the resident swT[i, ot, ic, o] (contiguous per-ot writes).
  - x pipeline: plain f32 SWDGE loads on the (otherwise idle) GpSimd
    queue — its in-order blocking on pool slots then gates nothing
    else — cast f32 -> bf16 (DVE for the first block, ScalarE once
    weight prep has drained), batched xbar DMA-transpose -> xT.
  - matmuls run in 8-token-tile blocks x 4 output-chunk waves so the
    first waves depend only on the first weight tiles: out[128t, 512o]
    accumulates 16 bf16 matmuls (xT chunk stationary, swT moving) in
    PSUM, DVE copies PSUM->SBUF, per-chunk DMA stores.

All xbar transposes issue from the Sync engine only — concurrent
DMA_TRANSPOSE from two engines corrupts the shared xbar. The casting
SWDGE DMA path is avoided (it moves only ~80 GB/s; plain SWDGE does
~276 GB/s). The tensor engine runs just the 2048 N=512 matmuls
(~213 ns each warm).
"""

import sys

if "/opt/trn_rl_repo" not in sys.path:
    sys.path.insert(0, "/opt/trn_rl_repo")

import numpy as np

T, I, O = 32768, 2048, 2048
NCORES = 8
TL = T // NCORES  # tokens per core

_NC = None


def _build():
    import concourse.bacc as bacc
    import concourse.mybir as mybir
    from concourse import tile
    from contextlib import ExitStack

    f32 = mybir.dt.float32
    bf16 = mybir.dt.bfloat16

    IC = I // 128  # i-chunks (contraction)
    OT = O // 128  # weight row tiles
    NT = TL // 128  # token tiles per core
    OCW = 512  # matmul moving free dim
    NOC = O // OCW
    BLK = 8  # token tiles per block
    NBLK = NT // BLK

    nc = bacc.Bacc("TRN2", target_bir_lowering=False, debug=False, num_devices=NCORES)
    x = nc.dram_tensor("x", [TL, I], f32, kind="ExternalInput")
    w = nc.dram_tensor("weight", [O, I], f32, kind="ExternalInput")
    out = nc.dram_tensor("out", [TL, O], f32, kind="ExternalOutput")

    with tile.TileContext(nc) as tc, ExitStack() as ctx:
        # sign(W).T resident: swT[i_p, ot, ic, o_l] =
        # sign(W)[128*ot + o_l, 128*ic + i_p]
        swt_pool = ctx.enter_context(tc.tile_pool(name="swt", bufs=1))
        swT = swt_pool.tile([128, OT, IC, 128], bf16)

        wprep = ctx.enter_context(tc.tile_pool(name="wprep", bufs=1))
        w_f32 = [
            wprep.tile([128, I], f32, tag="w_f32", name=f"w_f32_{ot}", bufs=3)
            for ot in range(OT)
        ]
        w_sgn = [
            wprep.tile([128, I], bf16, tag="w_sgn", name=f"w_sgn_{ot}", bufs=3)
            for ot in range(OT)
        ]
        # interleave loads and signs so the in-order Scalar stream never
        # waits on a pool slot whose release is behind it
        for ot in range(3):
            nc.scalar.dma_start(w_f32[ot][:], w[128 * ot : 128 * (ot + 1), :])
        for ot in range(OT):
            nc.scalar.activation(
                w_sgn[ot][:], w_f32[ot][:], mybir.ActivationFunctionType.Sign
            )
            if ot + 3 < OT:
                nc.scalar.dma_start(
                    w_f32[ot + 3][:], w[128 * (ot + 3) : 128 * (ot + 4), :]
                )

        xpool = ctx.enter_context(tc.tile_pool(name="xpool", bufs=1))
        xtpool = ctx.enter_context(tc.tile_pool(name="xtpool", bufs=10))
        opool = ctx.enter_context(tc.tile_pool(name="opool", bufs=6))
        psum_mm = ctx.enter_context(tc.tile_pool(name="psum_mm", bufs=4, space="PSUM"))

        # all x tile loads on the GpSimd (SWDGE) queue; pool-slot waits
        # block only this engine
        x_f32 = []
        for tt in range(NT):
            xf = xpool.tile([128, I], f32, tag="x_f32", name=f"x_f32_{tt}", bufs=3)
            nc.gpsimd.dma_start(xf[:], x[128 * tt : 128 * (tt + 1), :])
            x_f32.append(xf)

        # casts: DVE for block 0 (Scalar is busy with weight prep),
        # Scalar for the rest (DVE then only does PSUM copies)
        x_bf = []
        for tt in range(NT):
            xb = xpool.tile([128, I], bf16, tag="x_bf", name=f"x_bf_{tt}", bufs=3)
            x_bf.append(xb)
        for tt in range(BLK):
            nc.vector.tensor_copy(x_bf[tt][:], x_f32[tt][:])
        for tt in range(BLK, NT):
            nc.scalar.copy(x_bf[tt][:], x_f32[tt][:])

        xT = [None] * NT
        for blk in range(NBLK):
            tts = range(blk * BLK, (blk + 1) * BLK)
            if blk == 0:
                for ot in range(4):
                    nc.sync.dma_start(swT[:, ot, :, :], w_sgn[ot][:], transpose=True)
            for tt in tts:
                xT[tt] = xtpool.tile([128, IC, 128], bf16, tag="xT", name=f"xT_{tt}")
                nc.sync.dma_start(xT[tt][:], x_bf[tt][:], transpose=True)
            if blk == 0:
                for ot in range(4, OT):
                    nc.sync.dma_start(swT[:, ot, :, :], w_sgn[ot][:], transpose=True)

            for oc in range(NOC):
                for tt in tts:
                    acc = psum_mm.tile(
                        [128, OCW], f32, tag="acc", name=f"acc_{tt}_{oc}"
                    )
                    for ic in range(IC):
                        nc.tensor.matmul(
                            acc[:],
                            xT[tt][:, ic, :],
                            swT[:, 4 * oc : 4 * (oc + 1), ic, :],
                            start=(ic == 0),
                            stop=(ic == IC - 1),
                        )
                    o_ch = opool.tile([128, OCW], f32, tag="o_ch")
                    nc.vector.tensor_copy(o_ch[:], acc[:])
                    nc.sync.dma_start(
                        out[128 * tt : 128 * (tt + 1), OCW * oc : OCW * (oc + 1)],
                        o_ch[:],
                    )

    nc.compile()
    return nc


def _get_nc():
    global _NC
    if _NC is None:
        _NC = _build()
    return _NC


def _in_maps(x, w):
    x = np.ascontiguousarray(np.asarray(x, dtype=np.float32))
    w = np.ascontiguousarray(np.asarray(w, dtype=np.float32))
    assert x.shape == (T, I) and w.shape == (O, I)
    return [
        {"x": x[c * TL : (c + 1) * TL], "weight": w} for c in range(NCORES)
    ]


def kernel(**inputs):
    from concourse.bass_utils import run_bass_kernel_spmd

    nc = _get_nc()
    res = run_bass_kernel_spmd(
        nc, _in_maps(inputs["x"], inputs["weight"]), core_ids=list(range(NCORES))
    )
    return np.concatenate([r["out"] for r in res.results], axis=0)


# revision 11
# speedup vs baseline: 1.0837x; 1.0029x over previous
"""Trainium2 Bass kernel for BinaryLinear: out = x @ sign(W).T

Shapes (hardcoded): x [32768, 2048] f32, weight [2048, 2048] f32,
out [32768, 2048] f32.

Strategy: data-parallel over 8 NeuronCores — shard the token axis
(4096 tokens/core), replicate the weight. Per core:
  - weight prep: f32 loads on the Scalar HWDGE queue interleaved with
    Sign activations (f32 -> bf16), then batched xbar DMA-transposes
    into the resident swT[i, ot, ic, o] (contiguous per-ot writes).
  - x pipeline: f32 HWDGE loads (Sync for the first block, Scalar —
    interleaved with its ACT casts — once weight prep is done), cast
    f32 -> bf16, batched xbar DMA-transpose -> xT[i, ic, t].
  - matmuls run in 8-token-tile blocks x 4 output-chunk waves so the
    first waves depend only on the first weight tiles: out[128t, 512o]
    accumulates 16 bf16 matmuls (xT chunk stationary, swT moving) in
    PSUM, DVE copies PSUM->SBUF, per-chunk DMA stores. The next
    block's transposes are issued between waves 2 and 3 so block
    transitions cost the tensor engine nothing.

All xbar transposes issue from the Sync engine only — concurrent
DMA_TRANSPOSE from two engines corrupts the shared xbar. The Sync
prologue order (wT0,xT0,...,wT3,xT3 first) is pinned with explicit
scheduler dependencies: the Tile cost model otherwise reorders it and
delays the first matmul by ~70 us. The tensor engine runs just the
2048 N=512 matmuls (~213 ns each warm).
"""

import sys

if "/opt/trn_rl_repo" not in sys.path:
    sys.path.insert(0, "/opt/trn_rl_repo")

import numpy as np

T, I, O = 32768, 2048, 2048
NCORES = 8
TL = T // NCORES  # tokens per core

_NC = None


def _build():
    import concourse.bacc as bacc
    import concourse.mybir as mybir
    from concourse import tile
    from contextlib import ExitStack

    f32 = mybir.dt.float32
    bf16 = mybir.dt.bfloat16

    IC = I // 128  # i-chunks (contraction)
    OT = O // 128  # weight row tiles
    NT = TL // 128  # token tiles per core
    OCW = 512  # matmul moving free dim
    NOC = O // OCW
    BLK = 8  # token tiles per block
    NBLK = NT // BLK

    nc = bacc.Bacc("TRN2", target_bir_lowering=False, debug=False, num_devices=NCORES)
    x = nc.dram_tensor("x", [TL, I], f32, kind="ExternalInput")
    w = nc.dram_tensor("weight", [O, I], f32, kind="ExternalInput")
    out = nc.dram_tensor("out", [TL, O], f32, kind="ExternalOutput")

    with tile.TileContext(nc) as tc, ExitStack() as ctx:
        # sign(W).T resident: swT[i_p, ot, ic, o_l] =
        # sign(W)[128*ot + o_l, 128*ic + i_p]
        swt_pool = ctx.enter_context(tc.tile_pool(name="swt", bufs=1))
        swT = swt_pool.tile([128, OT, IC, 128], bf16)

        wprep = ctx.enter_context(tc.tile_pool(name="wprep", bufs=1))
        w_f32 = [
            wprep.tile([128, I], f32, tag="w_f32", name=f"w_f32_{ot}", bufs=3)
            for ot in range(OT)
        ]
        w_sgn = [
            wprep.tile([128, I], bf16, tag="w_sgn", name=f"w_sgn_{ot}", bufs=3)
            for ot in range(OT)
        ]
        # interleave loads and signs so the in-order Scalar stream never
        # waits on a pool slot whose release is behind it
        for ot in range(3):
            nc.scalar.dma_start(w_f32[ot][:], w[128 * ot : 128 * (ot + 1), :])
        for ot in range(OT):
            nc.scalar.activation(
                w_sgn[ot][:], w_f32[ot][:], mybir.ActivationFunctionType.Sign
            )
            if ot + 3 < OT:
                nc.scalar.dma_start(
                    w_f32[ot + 3][:], w[128 * (ot + 3) : 128 * (ot + 4), :]
                )

        xpool = ctx.enter_context(tc.tile_pool(name="xpool", bufs=1))
        xtpool = ctx.enter_context(tc.tile_pool(name="xtpool", bufs=8))
        opool = ctx.enter_context(tc.tile_pool(name="opool", bufs=6))
        psum_mm = ctx.enter_context(tc.tile_pool(name="psum_mm", bufs=4, space="PSUM"))

        x_f32 = [None] * NT
        x_bf = [None] * NT
        xT = [None] * NT
        def load_x(tt, eng):
            x_f32[tt] = xpool.tile(
                [128, I], f32, tag="x_f32", name=f"x_f32_{tt}", bufs=4
            )
            eng.dma_start(x_f32[tt][:], x[128 * tt : 128 * (tt + 1), :])

        def cast_x(tt, eng):
            x_bf[tt] = xpool.tile(
                [128, I], bf16, tag="x_bf", name=f"x_bf_{tt}", bufs=3
            )
            eng(x_bf[tt][:], x_f32[tt][:])

        def transpose_x(tt):
            xT[tt] = xtpool.tile([128, IC, 128], bf16, tag="xT", name=f"xT_{tt}")
            nc.sync.dma_start(xT[tt][:], x_bf[tt][:], transpose=True)

        def transpose_w(ot):
            nc.sync.dma_start(swT[:, ot, :, :], w_sgn[ot][:], transpose=True)

        # prologue: block-0 x loads on the idle GpSimd (plain SWDGE) queue
        for tt in range(BLK):
            load_x(tt, nc.gpsimd)
        for tt in range(BLK):
            cast_x(tt, nc.vector.tensor_copy)
        for k in range(4):
            transpose_w(k)
            transpose_x(k)
        for tt in range(4, BLK):
            transpose_x(tt)
        for ot in range(4, OT):
            transpose_w(ot)

        # x loads + casts for blocks 1..3 ride the Scalar queue, which
        # drains after weight prep; interleaved so slot waits can't
        # deadlock the in-order stream (bufs=3 on x_f32/x_bf)
        for tt in range(BLK, BLK + 2):
            load_x(tt, nc.scalar)
        for tt in range(BLK, NT):
            cast_x(tt, nc.scalar.copy)
            if tt + 2 < NT:
                load_x(tt + 2, nc.scalar)

        for blk in range(NBLK):
            tts = range(blk * BLK, (blk + 1) * BLK)
            nxt = range((blk + 1) * BLK, (blk + 2) * BLK) if blk + 1 < NBLK else []
            for oc in range(NOC):
                for tt in tts:
                    acc = psum_mm.tile(
                        [128, OCW], f32, tag="acc", name=f"acc_{tt}_{oc}"
                    )
                    for ic in range(IC):
                        nc.tensor.matmul(
                            acc[:],
                            xT[tt][:, ic, :],
                            swT[:, 4 * oc : 4 * (oc + 1), ic, :],
                            start=(ic == 0),
                            stop=(ic == IC - 1),
                        )
                    o_ch = opool.tile([128, OCW], f32, tag="o_ch")
                    nc.vector.tensor_copy(o_ch[:], acc[:])
                    nc.sync.dma_start(
                        out[128 * tt : 128 * (tt + 1), OCW * oc : OCW * (oc + 1)],
                        o_ch[:],
                    )
                if oc == 2:
                    for tt in nxt:
                        transpose_x(tt)

    nc.compile()
    return nc


def _get_nc():
    global _NC
    if _NC is None:
        _NC = _build()
    return _NC


def _in_maps(x, w):
    x = np.ascontiguousarray(np.asarray(x, dtype=np.float32))
    w = np.ascontiguousarray(np.asarray(w, dtype=np.float32))
    assert x.shape == (T, I) and w.shape == (O, I)
    return [
        {"x": x[c * TL : (c + 1) * TL], "weight": w} for c in range(NCORES)
    ]


def kernel(**inputs):
    from concourse.bass_utils import run_bass_kernel_spmd

    nc = _get_nc()
    res = run_bass_kernel_spmd(
        nc, _in_maps(inputs["x"], inputs["weight"]), core_ids=list(range(NCORES))
    )
    return np.concatenate([r["out"] for r in res.results], axis=0)


# revision 12
# speedup vs baseline: 1.3675x; 1.2619x over previous
"""Trainium2 Bass kernel for BinaryLinear: out = x @ sign(W).T

Shapes (hardcoded): x [32768, 2048] f32, weight [2048, 2048] f32,
out [32768, 2048] f32.

Strategy: data-parallel over 8 NeuronCores — shard the 32768-token
axis (4096 tokens/core) and replicate the weight. The sharding step on
the host also picks the device-friendly layouts (pure data movement —
all arithmetic stays on device):
  - x is fed per-core as xt[tt, i_p, ic, t_l] = x[128*tt + t_l,
    128*ic + i_p]: each token tile is one contiguous 1 MB load whose
    SBUF image is directly the pre-transposed stationary operand the
    PE wants (the systolic array contracts over the partition axis).
  - weight is fed as W.T [in, out] so sign(W).T is produced on-chip by
    a single ScalarE Sign activation pass (f32 -> bf16) per 128-row
    tile, no transposes.

Per core: weight tiles load on the Scalar HWDGE queue interleaved with
Sign activations; x tiles load on the Sync queue and are cast
f32 -> bf16 by DVE; each token tile then runs 4 output chunks x 16
contraction chunks of bf16 matmuls (xT chunk stationary, sign-weight
moving, N=512) accumulated in PSUM f32, DVE copies PSUM -> SBUF, and
one 1 MB store per token tile writes the f32 result. The tensor
engine runs only the 2048 matmuls per core (~213 ns each warm, the
compute roofline); DMA moves 80 MB/core, well under its ~360 GB/s.
"""

import sys

if "/opt/trn_rl_repo" not in sys.path:
    sys.path.insert(0, "/opt/trn_rl_repo")

import numpy as np

T, I, O = 32768, 2048, 2048
NCORES = 8
TL = T // NCORES  # tokens per core

_NC = None


def _build():
    import concourse.bacc as bacc
    import concourse.mybir as mybir
    from concourse import tile
    from contextlib import ExitStack

    f32 = mybir.dt.float32
    bf16 = mybir.dt.bfloat16

    IC = I // 128  # i-chunks (contraction)
    NT = TL // 128  # token tiles per core
    OCW = 512  # matmul moving free dim
    NOC = O // OCW

    nc = bacc.Bacc("TRN2", target_bir_lowering=False, debug=False, num_devices=NCORES)
    xt = nc.dram_tensor("xt", [NT, 128, IC, 128], f32, kind="ExternalInput")
    wt = nc.dram_tensor("wt", [I, O], f32, kind="ExternalInput")
    out = nc.dram_tensor("out", [TL, O], f32, kind="ExternalOutput")

    with tile.TileContext(nc) as tc, ExitStack() as ctx:
        # sign(W).T resident in SBUF as IC tiles of [128 i, O] bf16
        swt_pool = ctx.enter_context(tc.tile_pool(name="swt", bufs=1))
        swT = [swt_pool.tile([128, O], bf16, name=f"swT{ic}") for ic in range(IC)]

        wprep = ctx.enter_context(tc.tile_pool(name="wprep", bufs=1))
        w_f32 = [
            wprep.tile([128, O], f32, tag="w_f32", name=f"w_f32_{ic}", bufs=3)
            for ic in range(IC)
        ]
        # interleave loads and signs so the in-order Scalar stream never
        # waits on a pool slot whose release is behind it
        for ic in range(3):
            nc.scalar.dma_start(w_f32[ic][:], wt[128 * ic : 128 * (ic + 1), :])
        for ic in range(IC):
            nc.scalar.activation(
                swT[ic][:], w_f32[ic][:], mybir.ActivationFunctionType.Sign
            )
            if ic + 3 < IC:
                nc.scalar.dma_start(
                    w_f32[ic + 3][:], wt[128 * (ic + 3) : 128 * (ic + 4), :]
                )

        xpool = ctx.enter_context(tc.tile_pool(name="xpool", bufs=4))
        opool = ctx.enter_context(tc.tile_pool(name="opool", bufs=3))
        psum_mm = ctx.enter_context(tc.tile_pool(name="psum_mm", bufs=4, space="PSUM"))

        for tt in range(NT):
            x_f32 = xpool.tile([128, IC, 128], f32, tag="x_f32")
            nc.sync.dma_start(x_f32[:], xt[tt])
            xT = xpool.tile([128, IC, 128], bf16, tag="xT")
            nc.vector.tensor_copy(xT[:], x_f32[:])

            o_sb = opool.tile([128, O], f32, tag="o_sb")
            for oc in range(NOC):
                acc = psum_mm.tile([128, OCW], f32, tag="acc", name=f"acc_{tt}_{oc}")
                for ic in range(IC):
                    nc.tensor.matmul(
                        acc[:],
                        xT[:, ic, :],
                        swT[ic][:, OCW * oc : OCW * (oc + 1)],
                        start=(ic == 0),
                        stop=(ic == IC - 1),
                    )
                nc.vector.tensor_copy(o_sb[:, OCW * oc : OCW * (oc + 1)], acc[:])
            nc.sync.dma_start(out[128 * tt : 128 * (tt + 1), :], o_sb[:])

    nc.compile()
    return nc


def _get_nc():
    global _NC
    if _NC is None:
        _NC = _build()
    return _NC


def _in_maps(x, w):
    x = np.asarray(x, dtype=np.float32)
    w = np.asarray(w, dtype=np.float32)
    assert x.shape == (T, I) and w.shape == (O, I)
    # xt[tt, i_p, ic, t_l] = x[128*tt + t_l, 128*ic + i_p]
    xt = np.ascontiguousarray(
        x.reshape(T // 128, 128, I // 128, 128).transpose(0, 3, 2, 1)
    )
    wt = np.ascontiguousarray(w.T)
    ntl = TL // 128  # token tiles per core
    return [
        {"xt": xt[c * ntl : (c + 1) * ntl], "wt": wt} for c in range(NCORES)
    ]


def kernel(**inputs):
    from concourse.bass_utils import run_bass_kernel_spmd

    nc = _get_nc()
    res = run_bass_kernel_spmd(
        nc, _in_maps(inputs["x"], inputs["weight"]), core_ids=list(range(NCORES))
    )
    return np.concatenate([r["out"] for r in res.results], axis=0)


# revision 13
# speedup vs baseline: 1.3754x; 1.0058x over previous
"""Trainium2 Bass kernel for BinaryLinear: out = x @ sign(W).T

Shapes (hardcoded): x [32768, 2048] f32, weight [2048, 2048] f32,
out [32768, 2048] f32.

Strategy: data-parallel over 8 NeuronCores — shard the 32768-token
axis (4096 tokens/core) and replicate the weight. The sharding step on
the host also picks the device-friendly layouts (pure data movement —
all arithmetic stays on device):
  - x is fed per-core as xt[tt, i_p, ic, t_l] = x[128*tt + t_l,
    128*ic + i_p]: each token tile is one contiguous 1 MB load whose
    SBUF image is directly the pre-transposed stationary operand the
    PE wants (the systolic array contracts over the partition axis).
  - weight is fed as W.T [in, out] so sign(W).T is produced on-chip by
    a single ScalarE Sign activation pass (f32 -> bf16) per 128-row
    tile, no transposes.

Per core: weight tiles load on the Scalar HWDGE queue interleaved with
Sign activations; x tiles load on the Sync queue and are cast
f32 -> bf16 by DVE; each token tile then runs 4 output chunks x 16
contraction chunks of bf16 matmuls (xT chunk stationary, sign-weight
moving, N=512) accumulated in PSUM f32, DVE copies PSUM -> SBUF, and
one 1 MB store per token tile writes the f32 result. The tensor
engine runs only the 2048 matmuls per core (~213 ns each warm, the
compute roofline); DMA moves 80 MB/core, well under its ~360 GB/s.
"""

import sys

if "/opt/trn_rl_repo" not in sys.path:
    sys.path.insert(0, "/opt/trn_rl_repo")

import numpy as np

T, I, O = 32768, 2048, 2048
NCORES = 8
TL = T // NCORES  # tokens per core

_NC = None


def _build():
    import concourse.bacc as bacc
    import concourse.mybir as mybir
    from concourse import tile
    from contextlib import ExitStack

    f32 = mybir.dt.float32
    bf16 = mybir.dt.bfloat16

    IC = I // 128  # i-chunks (contraction)
    NT = TL // 128  # token tiles per core
    OCW = 512  # matmul moving free dim
    NOC = O // OCW

    nc = bacc.Bacc("TRN2", target_bir_lowering=False, debug=False, num_devices=NCORES)
    xt = nc.dram_tensor("xt", [NT, 128, IC, 128], f32, kind="ExternalInput")
    wt = nc.dram_tensor("wt", [I, O], f32, kind="ExternalInput")
    out = nc.dram_tensor("out", [TL, O], f32, kind="ExternalOutput")

    with tile.TileContext(nc) as tc, ExitStack() as ctx:
        # sign(W).T resident in SBUF as IC tiles of [128 i, O] bf16
        swt_pool = ctx.enter_context(tc.tile_pool(name="swt", bufs=1))
        swT = [swt_pool.tile([128, O], bf16, name=f"swT{ic}") for ic in range(IC)]

        wprep = ctx.enter_context(tc.tile_pool(name="wprep", bufs=1))
        w_f32 = [
            wprep.tile([128, O], f32, tag="w_f32", name=f"w_f32_{ic}", bufs=4)
            for ic in range(IC)
        ]
        # weight loads split across both HWDGE queues (even ics on
        # Scalar, odd on Sync) so the 16 MB of W lands ~2x sooner; loads
        # interleave with signs so the in-order Scalar stream never
        # waits on a pool slot whose release is behind it
        def load_w(ic):
            eng = nc.scalar if ic % 2 == 0 else nc.sync
            eng.dma_start(w_f32[ic][:], wt[128 * ic : 128 * (ic + 1), :])

        for ic in range(4):
            load_w(ic)
        for ic in range(IC):
            nc.scalar.activation(
                swT[ic][:], w_f32[ic][:], mybir.ActivationFunctionType.Sign
            )
            if ic + 4 < IC:
                load_w(ic + 4)

        xpool = ctx.enter_context(tc.tile_pool(name="xpool", bufs=4))
        opool = ctx.enter_context(tc.tile_pool(name="opool", bufs=3))
        psum_mm = ctx.enter_context(tc.tile_pool(name="psum_mm", bufs=4, space="PSUM"))

        for tt in range(NT):
            x_f32 = xpool.tile([128, IC, 128], f32, tag="x_f32")
            nc.sync.dma_start(x_f32[:], xt[tt])
            xT = xpool.tile([128, IC, 128], bf16, tag="xT")
            nc.vector.tensor_copy(xT[:], x_f32[:])

            o_sb = opool.tile([128, O], f32, tag="o_sb")
            for oc in range(NOC):
                acc = psum_mm.tile([128, OCW], f32, tag="acc", name=f"acc_{tt}_{oc}")
                for ic in range(IC):
                    nc.tensor.matmul(
                        acc[:],
                        xT[:, ic, :],
                        swT[ic][:, OCW * oc : OCW * (oc + 1)],
                        start=(ic == 0),
                        stop=(ic == IC - 1),
                    )
                nc.vector.tensor_copy(o_sb[:, OCW * oc : OCW * (oc + 1)], acc[:])
            nc.sync.dma_start(out[128 * tt : 128 * (tt + 1), :], o_sb[:])

    nc.compile()
    return nc


def _get_nc():
    global _NC
    if _NC is None:
        _NC = _build()
    return _NC


def _in_maps(x, w):
    x = np.asarray(x, dtype=np.float32)
    w = np.asarray(w, dtype=np.float32)
    assert x.shape == (T, I) and w.shape == (O, I)
    # xt[tt, i_p, ic, t_l] = x[128*tt + t_l, 128*ic + i_p]
    xt = np.ascontiguousarray(
        x.reshape(T // 128, 128, I // 128, 128).transpose(0, 3, 2, 1)
    )
    wt = np.ascontiguousarray(w.T)
    ntl = TL // 128  # token tiles per core
    return [
        {"xt": xt[c * ntl : (c + 1) * ntl], "wt": wt} for c in range(NCORES)
    ]


def kernel(**inputs):
    from concourse.bass_utils import run_bass_kernel_spmd

    nc = _get_nc()
    res = run_bass_kernel_spmd(
        nc, _in_maps(inputs["x"], inputs["weight"]), core_ids=list(range(NCORES))
    )
    return np.concatenate([r["out"] for r in res.results], axis=0)
